# revision 36
# baseline (speedup 1.0000x reference)
"""Trainium2 Bass kernel for the (non-standard) MultiHeadAttention module.

Reference math (B=4, N=2048, E=512, H=8):
    q/k/v  = x @ W{q,k,v} + b          # (B, N, E*H)
    split:   head h takes columns h::H  -> per-head (N, E) matrices
    attT_h = (k_h^T @ q_h) * 1/sqrt(N) # (f, e) -- attention over the E axis
    A_h    = exp(attT_h)               # softmax numerator (no max-sub
                                       #  needed, logits are O(+-5))
    s_h[e] = sum_f A_h[f, e]
    out row n' = 4e + r gets  sum_hl (A_h^T/s_h) @ P_h + bp
      for h = 2r + hl  (consequence of the reference's raw
      (B,E,H,N)->(B,N,E*H) reshape before the output projection), where
    P_h    = v_h^T @ Wp_half(hl)

Key algebraic refactors (this module attends over the E axis and contracts
over n, so everything collapses into E x E space):
  * Gram matrix  X = x_b^T @ x_b  (E x E, once per core; only the upper
    block-triangle is computed, the rest comes from PE transposes):
      attT_h = Wk_h^T X Wq_h + (Wk_h^T xs) (x) bq_h
               + bk_h (x) (Wq_h^T xs + N bq_h),   xs = colsum(x_b)
    -- eliminates the q/k projections entirely.
  * (A @ v^T) @ Wp == A @ (v^T @ Wp) and
    v_h^T @ Wp_hl == Wv_h^T @ G_hl + bv_h (x) colsum(Wp_hl)  with
    G_hl = x_b^T @ Wp_hl computed once per core -- eliminates the v
    projection and the big P matmuls.
  * the bv (x) swp rank-1 rides the DVE PSUM->SBUF copy
    (scalar_tensor_tensor), not the PE; bp is added once via the final
    out DVE op (A^T ones * r == 1 makes the old P-resident bp/2 exact).
  * softmax normalization at the very end: out = U0*r0 + U1*r1 + bp,
    U_h = A_h^T @ P_h,  r_h = 1/s_h.
  * s_h computed with ones as the stationary operand (A moving, 4 wide
    matmuls) then transposed into e-partition layout -- avoids 16
    LDWEIGHTS-bound tiny matmuls per head.
  * no PE warm-up: the HW throttles the PE to ~half speed until ~13us
    after NEFF launch regardless of what runs, so pass 1 starts
    immediately and pays the ramp with useful work.

Measured variants (8-core axon TRN2, NTFF exec_time):
    129.0us  v1 baseline (warm-up + PE bias matmuls)
    123.0-123.6us  no warm-up, P-bias on DVE, bp in the out op, T1
             over 4 PSUM banks, split final store
    122.4us  + xn[0] split into column halves on two queues with
             chunk 0 m-descending (first matmul 10.3 -> 9.3us),
             + wk/wp1 DMA issues moved off the Scalar queue so they
             cannot stall the ACT engine's copies/exp
    122.1us  this version: + previous head's s emitted before P's
             second half (its mm-pool PSUM tiles drain during P, so
             the next head's T1 chains don't WAR-wait), + T1 SBUF
             tiles double-buffered.
             Remaining time: framework pre/postamble (~10us), 96.6us
             of bf16 PE issue, the ~13us half-clock launch window,
             ~8us of accumulation-group boundary latencies
    125.5us  attT bias moved to DVE stt too -- DVE PSUM reads (~740ns
             per [128,512]) back up the attT->exp chain; reverted
    125.7us  DMA-ring "warmer" transfers -- they only delay xn[0]
    145.2us  G split across core pairs + 2-core DRAM AllGather -- the
             collective costs 8-13us mesh latency + core skew, and
             hangs under single-core NTFF profiling; reverted

Everything runs in bf16 (inputs cast host-side; fp32 PSUM accumulate),
which keeps the PE at full speed and halves DMA + SBUF.

Sharding: 16 independent units (b, r), b in 0..3, r in 0..3; unit (b, r)
owns heads {2r, 2r+1} and produces output rows out[b, r::4, :].  Two units
per core, batch-major:  core c -> b = c//2, r in {2*(c%2), 2*(c%2)+1}.
No inter-core communication.  (A 2-core AllGather G-split was tried and
measured SLOWER: the DRAM collective costs 8-13us of mesh latency + core
skew, and hangs under single-core NTFF profiling.)
"""

import ml_dtypes
import numpy as np
from contextlib import ExitStack

import concourse.bass as bass
import concourse.mybir as mybir
import concourse.tile as tile
from concourse import bacc
from concourse.bass_utils import run_bass_kernel_spmd

BF16_NP = ml_dtypes.bfloat16

B, N, E, H = 4, 2048, 512, 8
NT = N // 128          # 16 contraction chunks of 128 over n
EB = E // 128          # 4 blocks of 128 over e/f
SCALE = float(1.0 / np.sqrt(np.float32(N)))
F32 = mybir.dt.float32
BF16 = mybir.dt.bfloat16
PSUM = bass.MemorySpace.PSUM

_CACHED_NC = None


def build_nc():
    nc = bacc.Bacc("TRN2", target_bir_lowering=False, debug=False)

    xn_d = nc.dram_tensor("xn", (N, E), BF16, kind="ExternalInput")
    wq_d = nc.dram_tensor("wq", (2, 2, 128, EB, E), BF16, kind="ExternalInput")
    wk_d = nc.dram_tensor("wk", (2, 2, 128, EB, E), BF16, kind="ExternalInput")
    wv_d = nc.dram_tensor("wv", (2, 2, 128, EB, E), BF16, kind="ExternalInput")
    wp_d = nc.dram_tensor("wp", (2, N, E), BF16, kind="ExternalInput")
    # biasx[p, u, hl, which, :]:
    #   which 0: p0 = Wk_h^T xs, p1 = bk_h          (attT stationary)
    #   which 1: p0 = bq_h, p1 = Wq_h^T xs + N bq_h (attT moving)
    biasx_d = nc.dram_tensor("biasx", (2, 2, 2, 2, E), BF16, kind="ExternalInput")
    # bv in per-partition layout for the DVE rank-1: bvt[128, u, hl, fb]
    bvt_d = nc.dram_tensor("bvt", (128, 2, 2, EB), BF16, kind="ExternalInput")
    # colsum(Wp_hl) replicated across partitions, and bp replicated
    swp_d = nc.dram_tensor("swp", (2, 128, E), BF16, kind="ExternalInput")
    bp_d = nc.dram_tensor("bp", (128, E), BF16, kind="ExternalInput")
    eye_d = nc.dram_tensor("eye", (128, 128), BF16, kind="ExternalInput")
    onescol_d = nc.dram_tensor("onescol", (128, 1), BF16, kind="ExternalInput")
    out_d = nc.dram_tensor("out", (2, E, E), BF16, kind="ExternalOutput")

    with tile.TileContext(nc) as tc, ExitStack() as ctx:
        consts = ctx.enter_context(tc.tile_pool(name="consts", bufs=1))
        stream = ctx.enter_context(tc.tile_pool(name="stream", bufs=4))
        wqkv_pool = ctx.enter_context(tc.tile_pool(name="wqkv", bufs=2))
        t1_pool = ctx.enter_context(tc.tile_pool(name="t1", bufs=2))
        a_pool = ctx.enter_context(tc.tile_pool(name="a", bufs=2))
        p_pool = ctx.enter_context(tc.tile_pool(name="p", bufs=2))
        o_pool = ctx.enter_context(tc.tile_pool(name="o", bufs=4))
        r_pool = ctx.enter_context(tc.tile_pool(name="r", bufs=2))
        mm_ps = ctx.enter_context(tc.tile_pool(name="mmps", bufs=2, space=PSUM))
        big_ps = ctx.enter_context(tc.tile_pool(name="bigps", bufs=1, space=PSUM))
        u_ps = ctx.enter_context(tc.tile_pool(name="ups", bufs=2, space=PSUM))

        # The whole wp0 half resident, first on the gpsimd queue: no buffer
        # rotation throttling the prefetch depth.
        wp_primed = {}
        for pn in range(NT):
            psl = slice(pn * 128, (pn + 1) * 128)
            w0 = stream.tile([128, E], BF16, tag="wp0", name=f"wp0p{pn}", bufs=16)
            nc.gpsimd.dma_start(out=w0[:], in_=wp_d.ap()[0, psl, :])
            wp_primed[pn] = w0

        # x (natural layout), resident: feeds both the X and G phases.
        # Split across the sync and scalar DMA queues, ahead of everything
        # else those queues carry.
        xn_sb = []
        for n in range(NT):
            t = consts.tile([128, E], BF16, tag=f"xn{n}", name=f"xn{n}")
            eng = nc.sync if n % 2 == 0 else nc.scalar
            if n == 0:
                # first chunk as two column halves on parallel queues: the
                # halves land ~0.8us sooner than one 128KB transfer, and
                # chunk 0's m=3/m=2 matmuls only need the second half
                nc.sync.dma_start(
                    out=t[:, 0 : E // 2],
                    in_=xn_d.ap()[0:128, 0 : E // 2],
                )
                nc.scalar.dma_start(
                    out=t[:, E // 2 :],
                    in_=xn_d.ap()[0:128, E // 2 :],
                )
            else:
                eng.dma_start(
                    out=t[:], in_=xn_d.ap()[n * 128 : (n + 1) * 128, :]
                )
            xn_sb.append(t)

        # ---- other resident constants (scalar queue, ahead of wp1) ----
        biasx_sb = consts.tile([2, 2, 2, 2, E], BF16, tag="biasx")
        nc.scalar.dma_start(out=biasx_sb[:], in_=biasx_d.ap())
        bvt_sb = consts.tile([128, 2, 2, EB], BF16, tag="bvt")
        nc.scalar.dma_start(out=bvt_sb[:], in_=bvt_d.ap())
        swp_sb = consts.tile([128, 2, E], BF16, tag="swp")
        nc.scalar.dma_start(out=swp_sb[:, 0, :], in_=swp_d.ap()[0])
        nc.scalar.dma_start(out=swp_sb[:, 1, :], in_=swp_d.ap()[1])
        bp_sb = consts.tile([128, E], BF16, tag="bp")
        nc.scalar.dma_start(out=bp_sb[:], in_=bp_d.ap())
        eye_sb = consts.tile([128, 128], BF16, tag="eye")
        nc.scalar.dma_start(out=eye_sb[:], in_=eye_d.ap())
        onescol_sb = consts.tile([128, 1], BF16, tag="onescol")
        nc.scalar.dma_start(out=onescol_sb[:], in_=onescol_d.ap())

        # ---- pass 1: X = x^T x (upper block-triangle only) + G0 = x^T Wp0,
        # one shared sweep over n so X is not xn-starved alone ----
        X_ps = big_ps.tile([128, EB, E], F32, tag="big")
        g_sb = [
            consts.tile([128, EB, E], BF16, tag=f"g{hl}", name=f"g{hl}")
            for hl in range(2)
        ]
        g0_slots = [
            mm_ps.tile([128, E], F32, tag="mm", name="g0a"),
            mm_ps.tile([128, E], F32, tag="mm", name="g0b"),
            u_ps.tile([128, E], F32, tag="u", name="g0c"),
            u_ps.tile([128, E], F32, tag="u", name="g0d"),
        ]
        gate_p1c10 = None
        for n in range(NT):
            wp0_sb = wp_primed[n]
            # X first: it only needs xn[n], buying wp0[n] arrival slack.
            # Chunk 0 runs m descending: m=3/m=2 touch only the second
            # column half of xn[0], which lands first (split DMA above).
            m_order = range(EB - 1, -1, -1) if n == 0 else range(EB)
            for m in m_order:
                msl = slice(m * 128, (m + 1) * 128)
                nc.tensor.matmul(
                    X_ps[:, m, m * 128 :],
                    xn_sb[n][:, msl],
                    xn_sb[n][:, m * 128 :],
                    start=n == 0,
                    stop=n == NT - 1,
                )
            for m in m_order:
                msl = slice(m * 128, (m + 1) * 128)
                g_bi = nc.tensor.matmul(
                    g0_slots[m][:],
                    xn_sb[n][:, msl],
                    wp0_sb[:],
                    start=n == 0,
                    stop=n == NT - 1,
                )
                if n == 6 and m == 0:
                    gate_p1c10 = g_bi.ins
        # wp1: the whole half resident up front (2MB), on the SYNC queue
        # (keeping DMA issues off the Scalar/ACT engine, which runs the
        # copies and exp), execution gated to pass-1 chunk 6 so it doesn't
        # steal early pass-1 bandwidth but is fully in SBUF before pass 2
        # needs it.
        wp1_primed = {}
        for pn in range(NT):
            psl = slice(pn * 128, (pn + 1) * 128)
            w1 = stream.tile([128, E], BF16, tag="wp1", name=f"wp1p{pn}", bufs=16)
            bi = nc.sync.dma_start(out=w1[:], in_=wp_d.ap()[1, psl, :])
            tile.add_dep_helper(bi.ins, gate_p1c10, reason="delay wp1")
            wp1_primed[pn] = w1
        X_sb = consts.tile([128, EB, E], BF16, tag="X")
        for m in range(EB):
            if m < 2:
                nc.scalar.activation(
                    out=X_sb[:, m, m * 128 :],
                    in_=X_ps[:, m, m * 128 :],
                    func=mybir.ActivationFunctionType.Copy,
                )
            else:
                nc.vector.tensor_copy(X_sb[:, m, m * 128 :], X_ps[:, m, m * 128 :])
            nc.vector.tensor_copy(g_sb[0][:, m, :], g0_slots[m][:])

        # ---- pass 2: G1 = x^T Wp1; the X lower-triangle transposes are
        # interleaved a few chunks in (their DVE source copies have
        # completed by then) ----
        g1_slots = [
            mm_ps.tile([128, E], F32, tag="mm", name="g1a"),
            mm_ps.tile([128, E], F32, tag="mm", name="g1b"),
            u_ps.tile([128, E], F32, tag="u", name="g1c"),
            u_ps.tile([128, E], F32, tag="u", name="g1d"),
        ]
        xtrans_ps = big_ps.tile([128, EB, E], BF16, tag="big", name="xtrans")
        trans_jobs = [(m, ec) for m in range(1, EB) for ec in range(m)]
        gate_p2start = None
        gate_gmid = None
        for n in range(NT):
            wp1_sb = wp1_primed[n]
            for m in range(EB):
                msl = slice(m * 128, (m + 1) * 128)
                g_bi = nc.tensor.matmul(
                    g1_slots[m][:],
                    xn_sb[n][:, msl],
                    wp1_sb[:],
                    start=n == 0,
                    stop=n == NT - 1,
                )
                if n == 0 and m == 0:
                    gate_p2start = g_bi.ins
                if n == NT // 2 and m == 0:
                    gate_gmid = g_bi.ins
            if 2 <= n < 2 + len(trans_jobs):
                tm, tec = trans_jobs[n - 2]
                tpo = xtrans_ps[:, tm, tec * 128 : (tec + 1) * 128]
                nc.tensor.transpose(
                    tpo, X_sb[:, tec, tm * 128 : (tm + 1) * 128], eye_sb[:]
                )
                nc.vector.tensor_copy(
                    X_sb[:, tm, tec * 128 : (tec + 1) * 128], tpo
                )
        for m in range(EB):
            if m < 2:
                nc.scalar.activation(
                    out=g_sb[1][:, m, :],
                    in_=g1_slots[m][:],
                    func=mybir.ActivationFunctionType.Copy,
                )
            else:
                nc.vector.tensor_copy(g_sb[1][:, m, :], g1_slots[m][:])

        gate_hist = [gate_p2start, gate_gmid]  # per-head early gates
        pending_s = None

        def emit_pending_s():
            nonlocal pending_s
            if pending_s is None:
                return
            A_sb, R_list = pending_s
            pending_s = None
            # s row = ones^T @ A  (A moving: only 4 cheap stationary loads)
            srow_ps = mm_ps.tile([1, E], F32, tag="mm", name="srow")
            for fc in range(EB):
                nc.tensor.matmul(
                    srow_ps[:],
                    onescol_sb[:],
                    A_sb[:, fc, :],
                    start=fc == 0,
                    stop=fc == EB - 1,
                )
            srow_sb = r_pool.tile([1, E], BF16, tag="srow")
            nc.vector.tensor_copy(srow_sb[:], srow_ps[:])
            # transpose 128-wide pieces into e-partition layout ([128, EB, 2]
            # keeps each bf16 column 4-byte aligned in PSUM)
            sT_ps = mm_ps.tile([128, EB, 2], BF16, tag="mm", name="sT")
            for eb in range(EB):
                nc.tensor.transpose(
                    sT_ps[:, eb, 0:1],
                    srow_sb[0:1, eb * 128 : (eb + 1) * 128],
                    eye_sb[0:1, 0:1],
                )
            r_sb = r_pool.tile([128, EB], F32, tag="r")
            nc.vector.reciprocal(out=r_sb[:], in_=sT_ps[:, :, 0])
            R_list.append(r_sb)

        for u in range(2):
            A_tiles, P_tiles, R_tiles = [], [], []
            for hl in range(2):
                # --- weights for head (u, hl), prefetch-gated ---
                wq_sb = wqkv_pool.tile([128, EB, E], BF16, tag="wq")
                wq_bi = nc.gpsimd.dma_start(out=wq_sb[:], in_=wq_d.ap()[u, hl])
                wv_sb = wqkv_pool.tile([128, EB, E], BF16, tag="wv")
                wv_bi = nc.gpsimd.dma_start(out=wv_sb[:], in_=wv_d.ap()[u, hl])
                # wk on sync, not scalar: a 0.6us DMA issue on the Scalar
                # queue would delay the ACT engine's exp / T1 copies
                wk_sb = wqkv_pool.tile([128, EB, E], BF16, tag="wk")
                wk_bi = nc.sync.dma_start(out=wk_sb[:], in_=wk_d.ap()[u, hl])
                gate = gate_hist[-2]  # two head-phases back
                for bi in (wq_bi, wv_bi, wk_bi):
                    tile.add_dep_helper(bi.ins, gate, reason="delay prefetch")

                # --- T1 = X @ Wq_h, one m-block per PSUM bank; copies split
                # over ACT and DVE so attT can chase them block-by-block.
                # The previous head's exp runs on ACT during these matmuls.
                T1_sbs = []
                t1_first = None
                t1_slots = [mm_ps, mm_ps, u_ps, u_ps]
                for m in range(EB):
                    msl = slice(m * 128, (m + 1) * 128)
                    t1_ps = t1_slots[m].tile(
                        [128, E], F32, tag="mm" if m < 2 else "u", name=f"t1p{m}"
                    )
                    for ec in range(EB):
                        bi = nc.tensor.matmul(
                            t1_ps[:],
                            X_sb[:, ec, msl],
                            wq_sb[:, ec, :],
                            start=ec == 0,
                            stop=ec == EB - 1,
                        )
                        t1_first = t1_first or bi
                    t1_sb = t1_pool.tile([128, E], BF16, tag=f"t1{m}")
                    if m < 2:
                        nc.scalar.activation(
                            out=t1_sb[:],
                            in_=t1_ps[:],
                            func=mybir.ActivationFunctionType.Copy,
                        )
                    else:
                        nc.vector.tensor_copy(t1_sb[:], t1_ps[:])
                    T1_sbs.append(t1_sb)
                gate_early = t1_first.ins

                # --- P_h = Wv_h^T @ G_hl (+ bv_h (x) swp_hl on the DVE) ---
                # (independent of T1/attT; covers the T1 copy latency)
                P_sb = p_pool.tile([128, EB, E], BF16, tag="p")

                def emit_p_group(fb):
                    fsl = slice(fb * 128, (fb + 1) * 128)
                    p_ps = u_ps.tile([128, E], F32, tag="u", name=f"pp{fb}")
                    for ec in range(EB):
                        nc.tensor.matmul(
                            p_ps[:],
                            wv_sb[:, ec, fsl],
                            g_sb[hl][:, ec, :],
                            start=ec == 0,
                            stop=ec == EB - 1,
                        )
                    # P = psum + bv (x) swp  (rank-1 on the DVE, not the PE)
                    nc.vector.scalar_tensor_tensor(
                        P_sb[:, fb, :],
                        swp_sb[:, hl, :],
                        bvt_sb[:, u, hl, fb : fb + 1],
                        p_ps[:],
                        op0=mybir.AluOpType.mult,
                        op1=mybir.AluOpType.add,
                    )

                emit_p_group(0)
                emit_p_group(1)

                # --- attT = Wk_h^T @ T1 + [hvec0;bk] (x) [bq;hvec1] ---
                attT_ps = big_ps.tile([128, EB, E], F32, tag="big")
                for fb in range(EB):
                    fsl = slice(fb * 128, (fb + 1) * 128)
                    for ec in range(EB):
                        nc.tensor.matmul(
                            attT_ps[:, fb, :],
                            wk_sb[:, ec, fsl],
                            T1_sbs[ec][:],
                            start=ec == 0,
                            stop=False,
                        )
                    nc.tensor.matmul(
                        attT_ps[:, fb, :],
                        biasx_sb[0:2, u, hl, 0, fsl],
                        biasx_sb[0:2, u, hl, 1, :],
                        start=False,
                        stop=True,
                    )

                # --- exp (softmax numerator, transposed layout) ---
                A_sb = a_pool.tile([128, EB, E], BF16, tag="a")
                for fb in range(EB):
                    nc.scalar.activation(
                        out=A_sb[:, fb, :],
                        in_=attT_ps[:, fb, :],
                        func=mybir.ActivationFunctionType.Exp,
                        scale=SCALE,
                    )

                # previous head's s BEFORE the second half of P: its
                # srow/sT PSUM tiles (mm pool) then drain during P's
                # ~3.4us, so the next head's T1 chains don't WAR-wait
                # on those banks
                emit_pending_s()
                emit_p_group(2)
                emit_p_group(3)
                P_tiles.append(P_sb)
                A_tiles.append(A_sb)
                pending_s = (A_sb, R_tiles)
                gate_hist.append(gate_early)

            # --- U_h = A_h^T @ P_h ; out = U0*r0 + U1*r1 + bp ---
            out_tiles = [
                o_pool.tile([128, E], BF16, tag="o", name=f"ot{u}_{i}")
                for i in range(EB)
            ]
            u_big = None
            for hl in range(2):
                if hl == 1:
                    # the attT big tile is fully read by exp ~2.6us into the
                    # U phase, so the second half's four U chains take its 4
                    # banks -- the mm/u pools then drain during hl1, and the
                    # next head's T1 chains start with no WAR wait
                    u_big = big_ps.tile([128, EB, E], F32, tag="big", name=f"ub{u}")
                for eb in range(EB):
                    if hl == 0 and eb == 2:
                        emit_pending_s()  # s of this unit's second head
                    esl = slice(eb * 128, (eb + 1) * 128)
                    if hl == 0:
                        # alternate pools: 4 effective PSUM slots so the
                        # chains don't WAR-stall on the DVE scale-out
                        u_pool = u_ps if eb % 2 == 0 else mm_ps
                        u_tile = u_pool.tile(
                            [128, E], F32, tag="u" if eb % 2 == 0 else "mm"
                        )
                    else:
                        u_tile = None
                    u_ap = u_tile[:] if hl == 0 else u_big[:, eb, :]
                    for fc in range(EB):
                        nc.tensor.matmul(
                            u_ap,
                            A_tiles[hl][:, fc, esl],
                            P_tiles[hl][:, fc, :],
                            start=fc == 0,
                            stop=fc == EB - 1,
                        )
                    if hl == 0:
                        nc.vector.scalar_tensor_tensor(
                            out_tiles[eb][:],
                            u_ap,
                            R_tiles[0][:, eb : eb + 1],
                            bp_sb[:],
                            op0=mybir.AluOpType.mult,
                            op1=mybir.AluOpType.add,
                        )
                    elif not (u == 1 and eb == EB - 1):
                        nc.vector.scalar_tensor_tensor(
                            out_tiles[eb][:],
                            u_ap,
                            R_tiles[1][:, eb : eb + 1],
                            out_tiles[eb][:],
                            op0=mybir.AluOpType.mult,
                            op1=mybir.AluOpType.add,
                        )
                        eng = nc.sync if eb % 2 == 0 else nc.gpsimd
                        eng.dma_start(
                            out=out_d.ap()[u, eb * 128 : (eb + 1) * 128, :],
                            in_=out_tiles[eb][:],
                        )
                    else:
                        # very last tile: halve the scale+store pipeline depth
                        # (sync + scalar queues; gpsimd issues slowly here)
                        for half, eng in ((0, nc.sync), (1, nc.scalar)):
                            hsl = slice(half * 256, (half + 1) * 256)
                            nc.vector.scalar_tensor_tensor(
                                out_tiles[eb][:, hsl],
                                u_big[:, eb, hsl],
                                R_tiles[1][:, eb : eb + 1],
                                out_tiles[eb][:, hsl],
                                op0=mybir.AluOpType.mult,
                                op1=mybir.AluOpType.add,
                            )
                            eng.dma_start(
                                out=out_d.ap()[
                                    u, eb * 128 : (eb + 1) * 128, hsl
                                ],
                                in_=out_tiles[eb][:, hsl],
                            )

    nc.compile()
    return nc


def _get_nc():
    global _CACHED_NC
    if _CACHED_NC is None:
        _CACHED_NC = build_nc()
    return _CACHED_NC


def make_in_maps(x, Wq, bq, Wk, bk, Wv, bv, Wp, bp):
    x = np.asarray(x, np.float32)
    Wq, Wk, Wv, Wp = (np.asarray(a, np.float32) for a in (Wq, Wk, Wv, Wp))
    bq, bk, bv, bp = (np.asarray(a, np.float32) for a in (bq, bk, bv, bp))
    wp_arr = np.ascontiguousarray(np.stack([Wp[:N], Wp[N:]])).astype(BF16_NP)
    swp = np.stack([Wp[:N].sum(0), Wp[N:].sum(0)])  # (2, E)
    swp_rep = np.ascontiguousarray(
        np.broadcast_to(swp[:, None, :], (2, 128, E))
    ).astype(BF16_NP)
    bp_rep = np.ascontiguousarray(np.broadcast_to(bp[None, :], (128, E))).astype(
        BF16_NP
    )
    in_maps = []
    for c in range(8):
        b = c // 2
        rs = [2 * (c % 2), 2 * (c % 2) + 1]
        heads = [[2 * r + hl for hl in range(2)] for r in rs]
        xs = x[b].sum(0)

        def tile_w(Wm, h):
            # (E, E) -> [p, t, e] with row t*128+p on partition p
            return Wm[:, h::H].reshape(EB, 128, E).transpose(1, 0, 2)

        wq_arr = np.ascontiguousarray(
            np.stack([[tile_w(Wq, h) for h in hu] for hu in heads])
        ).astype(BF16_NP)
        wk_arr = np.ascontiguousarray(
            np.stack([[tile_w(Wk, h) for h in hu] for hu in heads])
        ).astype(BF16_NP)
        wv_arr = np.ascontiguousarray(
            np.stack([[tile_w(Wv, h) for h in hu] for hu in heads])
        ).astype(BF16_NP)
        biasx = np.zeros((2, 2, 2, 2, E), np.float32)
        bvt = np.zeros((128, 2, 2, EB), np.float32)
        for ui, hu in enumerate(heads):
            for hli, h in enumerate(hu):
                biasx[0, ui, hli, 0] = Wk[:, h::H].T @ xs
                biasx[1, ui, hli, 0] = bk[h::H]
                biasx[0, ui, hli, 1] = bq[h::H]
                biasx[1, ui, hli, 1] = Wq[:, h::H].T @ xs + np.float32(N) * bq[h::H]
                bvt[:, ui, hli, :] = bv[h::H].reshape(EB, 128).T
        in_maps.append(
            {
                "xn": np.ascontiguousarray(x[b]).astype(BF16_NP),
                "wq": wq_arr,
                "wk": wk_arr,
                "wv": wv_arr,
                "wp": wp_arr,
                "biasx": biasx.astype(BF16_NP),
                "bvt": bvt.astype(BF16_NP),
                "swp": swp_rep,
                "bp": bp_rep,
                "eye": np.eye(128, dtype=BF16_NP),
                "onescol": np.ones((128, 1), BF16_NP),
            }
        )
    return in_maps


def assemble_out(results):
    out = np.empty((B, N, E), np.float32)
    for c in range(8):
        b = c // 2
        for ui in range(2):
            r = 2 * (c % 2) + ui
            out[b, r::4, :] = results[c]["out"][ui].astype(np.float32)
    return out


def run(inputs, trace=False, **spmd_kwargs):
    """Full pipeline; returns (output, BassKernelResults)."""
    nc = _get_nc()
    in_maps = make_in_maps(**inputs)
    res = run_bass_kernel_spmd(
        nc, in_maps, core_ids=list(range(8)), trace=trace, **spmd_kwargs
    )
    return assemble_out(res.results), res


def kernel(**inputs):
    out, _ = run(inputs)
    return out
